# revision 1
# baseline (speedup 1.0000x reference)
"""MoE (top-2 of 8 experts, swiglu MLP) on 8 Trainium2 NeuronCores.

Strategy: expert parallelism — core e owns expert e's weights.
 - Host: router (fp64 softmax/top-2), gather each expert's tokens,
   pre-tile weights into the layouts the PE consumes directly.
 - Device (per core, SPMD one NEFF): the three expert matmuls in fp32r
   (full-rate PE) + silu/mul, producing yT = (expert_e(x_e)).T.
 - Host: combine — scale rows by gating weight and scatter-add into the
   full output (the "unshard" of expert-sharded partials).

Shapes: T=8192 tokens, H=2048, F=1408, E=8, K=2. Capacity C=2100
(max tokens/expert for this input is ~2100; overflow beyond C falls
back to a tiny host-side matmul, normally unused).
"""

import numpy as np

T, H, E, K, F = 8192, 2048, 8, 2, 1408
C = 2100  # token capacity per expert (>= max tokens/expert for this input)
N_CORES = 8

# c-chunks: (col offset, width, stage-A nb passes, stage-B cb blocks).
# All matmul N-blocks >= 256 so fp32r runs at 1 cycle/row.
CHUNKS = [
    (0, 1024, [[(0, 512), (512, 512)]], [(0, 512), (512, 512)]),
    (
        1024,
        1076,
        [[(0, 512), (512, 308)], [(820, 256)]],
        [(0, 512), (512, 308), (820, 256)],
    ),
]

_compiled = None


def _build():
    from contextlib import ExitStack

    import concourse.mybir as mybir
    import concourse.tile as tile
    from concourse import bacc

    f32 = mybir.dt.float32
    f32r = mybir.dt.float32r

    nc = bacc.Bacc("TRN2", target_bir_lowering=False, debug=False, num_devices=N_CORES)
    xt = nc.dram_tensor("xt", [H, C], f32r, kind="ExternalInput").ap()
    gu = nc.dram_tensor("gu", [2, 11, 128, 2048], f32r, kind="ExternalInput").ap()
    dw = nc.dram_tensor("dw", [16, 128, 1408], f32r, kind="ExternalInput").ap()
    yt = nc.dram_tensor("yt", [H, C], f32, kind="ExternalOutput").ap()

    with tile.TileContext(nc) as tc:
        with ExitStack() as ctx:
            pool_xt = ctx.enter_context(tc.tile_pool(name="xt", bufs=16))
            pool_gu = ctx.enter_context(tc.tile_pool(name="gu", bufs=4))
            pool_dw = ctx.enter_context(tc.tile_pool(name="dw", bufs=3))
            pool_h = ctx.enter_context(tc.tile_pool(name="h", bufs=11))
            pool_sil = ctx.enter_context(tc.tile_pool(name="sil", bufs=2))
            pool_out = ctx.enter_context(tc.tile_pool(name="out", bufs=4))
            psA = ctx.enter_context(tc.tile_pool(name="psA", bufs=4, space="PSUM"))
            psB = ctx.enter_context(tc.tile_pool(name="psB", bufs=3, space="PSUM"))

            for c0, cc, passes, cbs in CHUNKS:
                # f=0 weights first (on the scalar queue, split into pieces
                # so several DMA engines move them in parallel) so the PE
                # can start as soon as the first xt tile lands
                gut0 = pool_gu.tile([128, 2048], f32r, tag="gut", name="gut")
                uut0 = pool_gu.tile([128, 2048], f32r, tag="uut", name="uut")
                # chunk 0 only: use the otherwise-idle scalar queue so these
                # don't compete with xt on sync; later chunks keep scalar
                # free for the output stores
                eng0 = nc.scalar if c0 == 0 else nc.sync
                for q in range(4):
                    eng0.dma_start(
                        gut0[:, q * 512 : (q + 1) * 512],
                        gu[0, 0, :, q * 512 : (q + 1) * 512],
                    )
                for q in range(4):
                    eng0.dma_start(
                        uut0[:, q * 512 : (q + 1) * 512],
                        gu[1, 0, :, q * 512 : (q + 1) * 512],
                    )

                # token activations for this chunk, H on partitions
                xts = []
                for hb in range(16):
                    xtile = pool_xt.tile([128, cc], f32r, tag="xtile")
                    nc.sync.dma_start(
                        xtile[:], xt[hb * 128 : (hb + 1) * 128, c0 : c0 + cc]
                    )
                    xts.append(xtile)

                # stage A: hT[f, c] = silu(gT) * uT, gT = gate_w.T @ x.T
                hts = []
                for f in range(11):
                    ht = pool_h.tile([128, cc], f32r, tag="ht")
                    hts.append(ht)
                    if f == 0:
                        gut, uut = gut0, uut0
                    else:
                        gut = pool_gu.tile([128, 2048], f32r, tag="gut", name="gut")
                        uut = pool_gu.tile([128, 2048], f32r, tag="uut", name="uut")
                        for q in range(2):
                            nc.sync.dma_start(
                                gut[:, q * 1024 : (q + 1) * 1024],
                                gu[0, f, :, q * 1024 : (q + 1) * 1024],
                            )
                        for q in range(2):
                            nc.sync.dma_start(
                                uut[:, q * 1024 : (q + 1) * 1024],
                                gu[1, f, :, q * 1024 : (q + 1) * 1024],
                            )
                    for nbs in passes:
                        pgs = []
                        for i, (o, w) in enumerate(nbs):
                            pg = psA.tile([128, w], f32, tag="psA", name=f"pg{i}")
                            pgs.append(pg)
                        for h in range(16):
                            for pg, (o, w) in zip(pgs, nbs):
                                nc.tensor.matmul(
                                    pg[:],
                                    gut[:, h * 128 : (h + 1) * 128],
                                    xts[h][:, o : o + w],
                                    start=(h == 0),
                                    stop=(h == 15),
                                )
                        pus = []
                        for i, (o, w) in enumerate(nbs):
                            pu = psA.tile([128, w], f32, tag="psA", name=f"pu{i}")
                            pus.append(pu)
                        for h in range(16):
                            for pu, (o, w) in zip(pus, nbs):
                                nc.tensor.matmul(
                                    pu[:],
                                    uut[:, h * 128 : (h + 1) * 128],
                                    xts[h][:, o : o + w],
                                    start=(h == 0),
                                    stop=(h == 15),
                                )
                        for pg, pu, (o, w) in zip(pgs, pus, nbs):
                            sil = pool_sil.tile([128, w], f32, tag="sil")
                            nc.scalar.activation(
                                sil[:], pg[:], mybir.ActivationFunctionType.Silu
                            )
                            nc.vector.tensor_mul(
                                hts[f][:, o : o + w], sil[:], pu[:]
                            )

                # stage B: yT[h, c] = down_w @ hT  (gating applied on host)
                for hb in range(16):
                    dwt = pool_dw.tile([128, 1408], f32r, tag="dwt")
                    for q in range(2):
                        nc.sync.dma_start(
                            dwt[:, q * 704 : (q + 1) * 704],
                            dw[hb, :, q * 704 : (q + 1) * 704],
                        )
                    for o, w in cbs:
                        po = psB.tile([128, w], f32, tag="psB", name="po")
                        for f in range(11):
                            nc.tensor.matmul(
                                po[:],
                                dwt[:, f * 128 : (f + 1) * 128],
                                hts[f][:, o : o + w],
                                start=(f == 0),
                                stop=(f == 10),
                            )
                        ot = pool_out.tile([128, w], f32, tag="ot")
                        nc.vector.tensor_copy(ot[:], po[:])
                        nc.scalar.dma_start(
                            yt[hb * 128 : (hb + 1) * 128, c0 + o : c0 + o + w], ot[:]
                        )
    nc.compile()
    return nc


def _get_compiled():
    global _compiled
    if _compiled is None:
        _compiled = _build()
    return _compiled


def _route(x, router_w):
    """fp64 router: returns per-expert (indices, gating weights)."""
    logits = x.astype(np.float64) @ router_w.astype(np.float64).T
    logits -= logits.max(axis=-1, keepdims=True)
    p = np.exp(logits)
    p /= p.sum(axis=-1, keepdims=True)
    top2 = np.argsort(-p, axis=-1)[:, :K]
    pv = np.take_along_axis(p, top2, axis=-1)
    wts = pv / (pv.sum(axis=-1, keepdims=True) + 1e-20)
    idxs, gws = [], []
    for e in range(E):
        tok, pos = np.nonzero(top2 == e)
        idxs.append(tok.astype(np.int64))
        gws.append(wts[tok, pos].astype(np.float32))
    return idxs, gws


def _tile_gu(wT):
    # gu[f_blk, k, hb*128+m] = wT[hb*128+k, f_blk*128+m]
    return (
        wT.reshape(16, 128, 11, 128)
        .transpose(2, 1, 0, 3)
        .reshape(11, 128, 2048)
        .copy()
    )


def _tile_dw(D):
    # dw[hb, k, f_blk*128+m] = D[hb*128+m, f_blk*128+k]
    return (
        D.reshape(16, 128, 11, 128).transpose(0, 3, 2, 1).reshape(16, 128, 1408).copy()
    )


def _swiglu_host(xg, gate, up, down):
    g = xg @ gate.T
    u = xg @ up.T
    h = (g / (1.0 + np.exp(-g))) * u
    return h @ down.T


def kernel(hidden_states, router_w, gate_w, up_w, down_w):
    from concourse import bass_utils

    x = np.ascontiguousarray(hidden_states.reshape(-1, H).astype(np.float32))
    idxs, gws = _route(x, router_w)

    in_maps = []
    spill = []  # (expert, token_indices) handled on host if capacity exceeded
    for e in range(E):
        idx = idxs[e]
        if len(idx) > C:
            spill.append((e, idx[C:], gws[e][C:]))
            idx = idx[:C]
        xt = np.zeros((H, C), dtype=np.float32)
        xt[:, : len(idx)] = x[idx].T
        gu = np.stack(
            [
                _tile_gu(gate_w[e].T.astype(np.float32)),
                _tile_gu(up_w[e].T.astype(np.float32)),
            ]
        )
        dw = _tile_dw(down_w[e].astype(np.float32))
        in_maps.append({"xt": xt, "gu": gu, "dw": dw})

    global _last_in_maps
    _last_in_maps = in_maps
    nc = _get_compiled()
    res = bass_utils.run_bass_kernel_spmd(
        nc, in_maps, core_ids=list(range(N_CORES))
    )

    out = np.zeros((T, H), dtype=np.float32)
    for e in range(E):
        # token indices are unique within one expert (a token's two experts
        # are distinct), so fancy-index += is an exact scatter-add
        idx = idxs[e][:C]
        w = gws[e][:C]
        y = res.results[e]["yt"][:, : len(idx)].T
        out[idx] += w[:, None] * y
    for e, idx, w in spill:
        y = _swiglu_host(x[idx], gate_w[e], up_w[e], down_w[e]).astype(np.float32)
        out[idx] += w[:, None] * y
    return out.reshape(hidden_states.shape).astype(np.float32)



# revision 2
# speedup vs baseline: 1.1551x; 1.1551x over previous
"""MoE (top-2 of 8 experts, swiglu MLP) on 8 Trainium2 NeuronCores.

Strategy: expert parallelism — core e owns expert e's weights.
 - Host: router (fp64 softmax/top-2), gather each expert's tokens,
   pre-tile weights into the layouts the PE consumes directly, cast to
   bf16 (fp32 PSUM accumulation on device keeps the error ~3.5e-3 in
   the max-over-global-max metric).
 - Device (per core, SPMD one NEFF): single chunk of C=2048 columns
   (exactly T*K/8 — perfectly balanced), all matmul N-blocks 512.
   Stage A: hT = silu(gate_w.T @ xT) * (up_w.T @ xT); stage B:
   yT = down_w @ hT. Weights stream once (no re-chunking).
 - Host: combine — scale rows by gating weight and scatter-add into
   the full output. Tokens beyond the 2048-capacity of an expert are
   computed on the host (103 token-expert pairs for this input).

Shapes: T=8192 tokens, H=2048, F=1408, E=8, K=2, C=2048.
"""

import numpy as np

T, H, E, K, F = 8192, 2048, 8, 2, 1408
C = 2048  # token capacity per expert; overflow falls back to host
N_CORES = 8
NCB = C // 512  # 512-wide column blocks

_compiled = None


def _build():
    from contextlib import ExitStack

    import concourse.mybir as mybir
    import concourse.tile as tile
    from concourse import bacc

    f32 = mybir.dt.float32
    bf16 = mybir.dt.bfloat16

    nc = bacc.Bacc("TRN2", target_bir_lowering=False, debug=False, num_devices=N_CORES)
    xt = nc.dram_tensor("xt", [H, C], bf16, kind="ExternalInput").ap()
    gu = nc.dram_tensor("gu", [2, 11, 128, 2048], bf16, kind="ExternalInput").ap()
    dw = nc.dram_tensor("dw", [16, 128, 1408], bf16, kind="ExternalInput").ap()
    yt = nc.dram_tensor("yt", [H, C], f32, kind="ExternalOutput").ap()

    with tile.TileContext(nc) as tc:
        with ExitStack() as ctx:
            pool_xt = ctx.enter_context(tc.tile_pool(name="xt", bufs=16))
            pool_gu = ctx.enter_context(tc.tile_pool(name="gu", bufs=2))
            pool_dw = ctx.enter_context(tc.tile_pool(name="dw", bufs=3))
            pool_h = ctx.enter_context(tc.tile_pool(name="h", bufs=11))
            pool_sil = ctx.enter_context(tc.tile_pool(name="sil", bufs=4))
            pool_out = ctx.enter_context(tc.tile_pool(name="out", bufs=4))
            ps = ctx.enter_context(tc.tile_pool(name="ps", bufs=8, space="PSUM"))

            # f=0 weights on the gpsimd (SWDGE) ring so they don't queue
            # behind the xt tiles on the two HWDGE rings
            gut0 = pool_gu.tile([128, 2048], bf16, tag="gut", name="gut")
            uut0 = pool_gu.tile([128, 2048], bf16, tag="uut", name="uut")
            nc.gpsimd.dma_start(gut0[:], gu[0, 0])
            nc.gpsimd.dma_start(uut0[:], gu[1, 0])

            # token activations, H on partitions; alternate the two HWDGE
            # rings so tiles land at twice the single-ring rate
            xts = []
            for hb in range(16):
                xtile = pool_xt.tile([128, C], bf16, tag="xtile")
                eng = nc.sync if hb % 2 == 0 else nc.scalar
                eng.dma_start(xtile[:], xt[hb * 128 : (hb + 1) * 128, :])
                xts.append(xtile)

            # stage A: hT[f, c] = silu(gT) * uT, gT = gate_w.T @ x.T
            hts = []
            for f in range(11):
                if f == 0:
                    gut, uut = gut0, uut0
                else:
                    gut = pool_gu.tile([128, 2048], bf16, tag="gut", name="gut")
                    uut = pool_gu.tile([128, 2048], bf16, tag="uut", name="uut")
                    nc.sync.dma_start(gut[:], gu[0, f])
                    nc.scalar.dma_start(uut[:], gu[1, f])
                ht = pool_h.tile([128, C], bf16, tag="ht")
                hts.append(ht)
                pgs = [ps.tile([128, 512], f32, tag="ps", name="pg") for _ in range(NCB)]
                for h in range(16):
                    for cb, pg in enumerate(pgs):
                        nc.tensor.matmul(
                            pg[:],
                            gut[:, h * 128 : (h + 1) * 128],
                            xts[h][:, cb * 512 : (cb + 1) * 512],
                            start=(h == 0),
                            stop=(h == 15),
                        )
                pus = [ps.tile([128, 512], f32, tag="ps", name="pu") for _ in range(NCB)]
                for h in range(16):
                    for cb, pu in enumerate(pus):
                        nc.tensor.matmul(
                            pu[:],
                            uut[:, h * 128 : (h + 1) * 128],
                            xts[h][:, cb * 512 : (cb + 1) * 512],
                            start=(h == 0),
                            stop=(h == 15),
                        )
                for cb, (pg, pu) in enumerate(zip(pgs, pus)):
                    sil = pool_sil.tile([128, 512], f32, tag="sil")
                    nc.scalar.activation(
                        sil[:], pg[:], mybir.ActivationFunctionType.Silu
                    )
                    nc.vector.tensor_mul(
                        ht[:, cb * 512 : (cb + 1) * 512], sil[:], pu[:]
                    )

            # stage B: yT[h, c] = down_w @ hT  (gating applied on host)
            for hb in range(16):
                dwt = pool_dw.tile([128, 1408], bf16, tag="dwt")
                nc.sync.dma_start(dwt[:], dw[hb])
                for cb in range(NCB):
                    po = ps.tile([128, 512], f32, tag="ps", name="po")
                    for f in range(11):
                        nc.tensor.matmul(
                            po[:],
                            dwt[:, f * 128 : (f + 1) * 128],
                            hts[f][:, cb * 512 : (cb + 1) * 512],
                            start=(f == 0),
                            stop=(f == 10),
                        )
                    ot = pool_out.tile([128, 512], f32, tag="ot")
                    nc.vector.tensor_copy(ot[:], po[:])
                    nc.scalar.dma_start(
                        yt[hb * 128 : (hb + 1) * 128, cb * 512 : (cb + 1) * 512],
                        ot[:],
                    )
    nc.compile()
    return nc


def _get_compiled():
    global _compiled
    if _compiled is None:
        _compiled = _build()
    return _compiled


def _route(x, router_w):
    """fp64 router: returns per-expert (indices, gating weights)."""
    logits = x.astype(np.float64) @ router_w.astype(np.float64).T
    logits -= logits.max(axis=-1, keepdims=True)
    p = np.exp(logits)
    p /= p.sum(axis=-1, keepdims=True)
    top2 = np.argsort(-p, axis=-1)[:, :K]
    pv = np.take_along_axis(p, top2, axis=-1)
    wts = pv / (pv.sum(axis=-1, keepdims=True) + 1e-20)
    idxs, gws = [], []
    for e in range(E):
        tok, pos = np.nonzero(top2 == e)
        idxs.append(tok.astype(np.int64))
        gws.append(wts[tok, pos].astype(np.float32))
    return idxs, gws


def _tile_gu(wT):
    # gu[f_blk, k, hb*128+m] = wT[hb*128+k, f_blk*128+m]
    return (
        wT.reshape(16, 128, 11, 128)
        .transpose(2, 1, 0, 3)
        .reshape(11, 128, 2048)
        .copy()
    )


def _tile_dw(D):
    # dw[hb, k, f_blk*128+m] = D[hb*128+m, f_blk*128+k]
    return (
        D.reshape(16, 128, 11, 128).transpose(0, 3, 2, 1).reshape(16, 128, 1408).copy()
    )


def _swiglu_host(xg, gate, up, down):
    g = xg @ gate.T
    u = xg @ up.T
    h = (g / (1.0 + np.exp(-g))) * u
    return h @ down.T


def kernel(hidden_states, router_w, gate_w, up_w, down_w):
    import ml_dtypes
    from concourse import bass_utils

    bf16 = ml_dtypes.bfloat16
    x = np.ascontiguousarray(hidden_states.reshape(-1, H).astype(np.float32))
    idxs, gws = _route(x, router_w)

    in_maps = []
    spill = []  # (expert, token_indices, weights) handled on host
    for e in range(E):
        idx = idxs[e]
        if len(idx) > C:
            spill.append((e, idx[C:], gws[e][C:]))
            idx = idx[:C]
        xt = np.zeros((H, C), dtype=bf16)
        xt[:, : len(idx)] = x[idx].T.astype(bf16)
        gu = np.stack(
            [
                _tile_gu(gate_w[e].T.astype(np.float32)),
                _tile_gu(up_w[e].T.astype(np.float32)),
            ]
        ).astype(bf16)
        dw = _tile_dw(down_w[e].astype(np.float32)).astype(bf16)
        in_maps.append({"xt": xt, "gu": gu, "dw": dw})

    global _last_in_maps
    _last_in_maps = in_maps
    nc = _get_compiled()
    res = bass_utils.run_bass_kernel_spmd(
        nc, in_maps, core_ids=list(range(N_CORES))
    )

    out = np.zeros((T, H), dtype=np.float32)
    for e in range(E):
        # token indices are unique within one expert (a token's two experts
        # are distinct), so fancy-index += is an exact scatter-add
        idx = idxs[e][:C]
        w = gws[e][:C]
        y = res.results[e]["yt"][:, : len(idx)].T
        out[idx] += w[:, None] * y
    for e, idx, w in spill:
        y = _swiglu_host(x[idx], gate_w[e], up_w[e], down_w[e]).astype(np.float32)
        out[idx] += w[:, None] * y
    return out.reshape(hidden_states.shape).astype(np.float32)


# revision 7
# speedup vs baseline: 1.1785x; 1.0203x over previous
"""MoE (top-2 of 8 experts, swiglu MLP) on 8 Trainium2 NeuronCores.

Strategy: expert parallelism — core e owns expert e's weights.
 - Host: router (fp64 softmax/top-2), gather each expert's tokens,
   pre-tile weights into the layouts the PE consumes directly, cast to
   bf16 (fp32 PSUM accumulation on device keeps the error ~3.5e-3 in
   the max-over-global-max metric).
 - Device (per core, SPMD one NEFF): single chunk of C=2048 columns
   (exactly T*K/8 — perfectly balanced), all matmul N-blocks 512.
   Stage A: hT = silu(gate_w.T @ xT) * (up_w.T @ xT); stage B:
   yT = down_w @ hT. Weights stream once (no re-chunking).
 - Host: combine — scale rows by gating weight and scatter-add into
   the full output. Tokens beyond the 2048-capacity of an expert are
   computed on the host (103 token-expert pairs for this input).

Shapes: T=8192 tokens, H=2048, F=1408, E=8, K=2, C=2048.
"""

import numpy as np

T, H, E, K, F = 8192, 2048, 8, 2, 1408
C = 2048  # token capacity per expert; overflow falls back to host
N_CORES = 8
NCB = C // 512  # 512-wide column blocks

_compiled = None


def _build():
    from contextlib import ExitStack

    import concourse.mybir as mybir
    import concourse.tile as tile
    from concourse import bacc

    f32 = mybir.dt.float32
    bf16 = mybir.dt.bfloat16

    nc = bacc.Bacc("TRN2", target_bir_lowering=False, debug=False, num_devices=N_CORES)
    xt = nc.dram_tensor("xt", [H, C], bf16, kind="ExternalInput").ap()
    gu = nc.dram_tensor("gu", [2, 11, 128, 2048], bf16, kind="ExternalInput").ap()
    dw = nc.dram_tensor("dw", [16, 128, 1408], bf16, kind="ExternalInput").ap()
    yt = nc.dram_tensor("yt", [H, C], f32, kind="ExternalOutput").ap()

    with tile.TileContext(nc) as tc:
        with ExitStack() as ctx:
            pool_xt = ctx.enter_context(tc.tile_pool(name="xt", bufs=32))
            pool_gu = ctx.enter_context(tc.tile_pool(name="gu", bufs=2))
            pool_dw = ctx.enter_context(tc.tile_pool(name="dw", bufs=3))
            pool_h = ctx.enter_context(tc.tile_pool(name="h", bufs=11))
            pool_sil = ctx.enter_context(tc.tile_pool(name="sil", bufs=4))
            pool_out = ctx.enter_context(tc.tile_pool(name="out", bufs=4))
            ps = ctx.enter_context(tc.tile_pool(name="ps", bufs=8, space="PSUM"))

            # f=0..2 weights on the gpsimd (SWDGE) ring so they don't queue
            # behind the xt tiles on the two HWDGE rings
            guts, uuts = {}, {}
            for f in range(3):
                guts[f] = pool_gu.tile([128, 2048], bf16, tag="gut", name="gut")
                uuts[f] = pool_gu.tile([128, 2048], bf16, tag="uut", name="uut")
                nc.gpsimd.dma_start(guts[f][:], gu[0, f])
                nc.gpsimd.dma_start(uuts[f][:], gu[1, f])

            # token activations, H on partitions. Half-tiles (cols 0:1024
            # first, then 1024:2048) alternating across the two HWDGE rings:
            # stage A's first pass consumes cols 0:1024 of each h-tile at
            # ~850ns each, matching the ~700ns DMA delivery rate, so the PE
            # never starves at startup.
            xts = [[None] * 16, [None] * 16]  # [half][hb] -> [128, 1024] tile
            for half in range(2):
                for hb in range(16):
                    xtile = pool_xt.tile([128, C // 2], bf16, tag="xtile")
                    xts[half][hb] = xtile
                    eng = nc.sync if hb % 2 == 0 else nc.scalar
                    eng.dma_start(
                        xtile[:],
                        xt[hb * 128 : (hb + 1) * 128, half * 1024 : (half + 1) * 1024],
                    )

            # stage A: hT[f, c] = silu(gT) * uT, gT = gate_w.T @ x.T
            # g and u interleaved per h-tile in column-block pairs, so each
            # xt (half-)tile is fully consumed as soon as it lands.
            hts = []
            for f in range(11):
                if f in guts:
                    gut, uut = guts[f], uuts[f]
                else:
                    gut = pool_gu.tile([128, 2048], bf16, tag="gut", name="gut")
                    uut = pool_gu.tile([128, 2048], bf16, tag="uut", name="uut")
                    nc.sync.dma_start(gut[:], gu[0, f])
                    nc.scalar.dma_start(uut[:], gu[1, f])
                ht = pool_h.tile([128, C], bf16, tag="ht")
                hts.append(ht)
                for part in range(2):
                    cbs = (0, 1)  # column blocks within this half's tiles
                    pgs = [ps.tile([128, 512], f32, tag="ps", name="pg") for _ in cbs]
                    pus = [ps.tile([128, 512], f32, tag="ps", name="pu") for _ in cbs]
                    for h in range(16):
                        for cb, pg in zip(cbs, pgs):
                            nc.tensor.matmul(
                                pg[:],
                                gut[:, h * 128 : (h + 1) * 128],
                                xts[part][h][:, cb * 512 : (cb + 1) * 512],
                                start=(h == 0),
                                stop=(h == 15),
                            )
                        for cb, pu in zip(cbs, pus):
                            nc.tensor.matmul(
                                pu[:],
                                uut[:, h * 128 : (h + 1) * 128],
                                xts[part][h][:, cb * 512 : (cb + 1) * 512],
                                start=(h == 0),
                                stop=(h == 15),
                            )
                    for cb, pg, pu in zip(cbs, pgs, pus):
                        gcb = 2 * part + cb
                        sil = pool_sil.tile([128, 512], f32, tag="sil")
                        nc.scalar.activation(
                            sil[:], pg[:], mybir.ActivationFunctionType.Silu
                        )
                        nc.vector.tensor_mul(
                            ht[:, gcb * 512 : (gcb + 1) * 512], sil[:], pu[:]
                        )

            # stage B: yT[h, c] = down_w @ hT  (gating applied on host)
            for hb in range(16):
                dwt = pool_dw.tile([128, 1408], bf16, tag="dwt")
                nc.sync.dma_start(dwt[:], dw[hb])
                for cb in range(NCB):
                    po = ps.tile([128, 512], f32, tag="ps", name="po")
                    for f in range(11):
                        nc.tensor.matmul(
                            po[:],
                            dwt[:, f * 128 : (f + 1) * 128],
                            hts[f][:, cb * 512 : (cb + 1) * 512],
                            start=(f == 0),
                            stop=(f == 10),
                        )
                    ot = pool_out.tile([128, 512], f32, tag="ot")
                    nc.vector.tensor_copy(ot[:], po[:])
                    nc.scalar.dma_start(
                        yt[hb * 128 : (hb + 1) * 128, cb * 512 : (cb + 1) * 512],
                        ot[:],
                    )
    nc.compile()
    return nc


def _get_compiled():
    global _compiled
    if _compiled is None:
        _compiled = _build()
    return _compiled


def _route(x, router_w):
    """fp64 router: returns per-expert (indices, gating weights)."""
    logits = x.astype(np.float64) @ router_w.astype(np.float64).T
    logits -= logits.max(axis=-1, keepdims=True)
    p = np.exp(logits)
    p /= p.sum(axis=-1, keepdims=True)
    top2 = np.argsort(-p, axis=-1)[:, :K]
    pv = np.take_along_axis(p, top2, axis=-1)
    wts = pv / (pv.sum(axis=-1, keepdims=True) + 1e-20)
    idxs, gws = [], []
    for e in range(E):
        tok, pos = np.nonzero(top2 == e)
        idxs.append(tok.astype(np.int64))
        gws.append(wts[tok, pos].astype(np.float32))
    return idxs, gws


def _tile_gu(wT):
    # gu[f_blk, k, hb*128+m] = wT[hb*128+k, f_blk*128+m]
    return (
        wT.reshape(16, 128, 11, 128)
        .transpose(2, 1, 0, 3)
        .reshape(11, 128, 2048)
        .copy()
    )


def _tile_dw(D):
    # dw[hb, k, f_blk*128+m] = D[hb*128+m, f_blk*128+k]
    return (
        D.reshape(16, 128, 11, 128).transpose(0, 3, 2, 1).reshape(16, 128, 1408).copy()
    )


def _swiglu_host(xg, gate, up, down):
    g = xg @ gate.T
    u = xg @ up.T
    h = (g / (1.0 + np.exp(-g))) * u
    return h @ down.T


def kernel(hidden_states, router_w, gate_w, up_w, down_w):
    import ml_dtypes
    from concourse import bass_utils

    bf16 = ml_dtypes.bfloat16
    x = np.ascontiguousarray(hidden_states.reshape(-1, H).astype(np.float32))
    idxs, gws = _route(x, router_w)

    in_maps = []
    spill = []  # (expert, token_indices, weights) handled on host
    for e in range(E):
        idx = idxs[e]
        if len(idx) > C:
            spill.append((e, idx[C:], gws[e][C:]))
            idx = idx[:C]
        xt = np.zeros((H, C), dtype=bf16)
        xt[:, : len(idx)] = x[idx].T.astype(bf16)
        gu = np.stack(
            [
                _tile_gu(gate_w[e].T.astype(np.float32)),
                _tile_gu(up_w[e].T.astype(np.float32)),
            ]
        ).astype(bf16)
        dw = _tile_dw(down_w[e].astype(np.float32)).astype(bf16)
        in_maps.append({"xt": xt, "gu": gu, "dw": dw})

    global _last_in_maps
    _last_in_maps = in_maps
    nc = _get_compiled()
    res = bass_utils.run_bass_kernel_spmd(
        nc, in_maps, core_ids=list(range(N_CORES))
    )

    out = np.zeros((T, H), dtype=np.float32)
    for e in range(E):
        # token indices are unique within one expert (a token's two experts
        # are distinct), so fancy-index += is an exact scatter-add
        idx = idxs[e][:C]
        w = gws[e][:C]
        y = res.results[e]["yt"][:, : len(idx)].T
        out[idx] += w[:, None] * y
    for e, idx, w in spill:
        y = _swiglu_host(x[idx], gate_w[e], up_w[e], down_w[e]).astype(np.float32)
        out[idx] += w[:, None] * y
    return out.reshape(hidden_states.shape).astype(np.float32)
